# revision 1
# baseline (speedup 1.0000x reference)
"""Causal self-attention (B=4, T=2048, D=1024, H=16) on 8 TRN2 NeuronCores.

Sharding: core i = (batch b = i//2, head-group g = i%2). Data parallel on B,
tensor parallel on heads (8 heads per group): qkv_proj columns and out_proj
rows split per head group. Each core computes a partial [D, T] output^T for
its batch; host sums the two group partials per batch, transposes, adds bias.

Per-core pipeline (all matmuls in float32r = FP22, full PE rate at N>=256):
  phase 1: x -> x^T via PE transpose; V = x@Wv (natural [t,d] + ones col);
           Q^T, K^T = (x@Wq)^T via transposed projection, bounced to DRAM.
  phase 2: per head pair p, per q-chunk qc (512), per k-tile j (128):
           S^T[k,q] = K^T.T @ Q^T (heads at partitions 0-63 / 64-127);
           one exp over both heads' strips (trimmed to the causal columns);
           triangle mask-mul on the diagonal 128-block; AV: psum[65,512] +=
           V'[k,d+1].T @ P^T accumulated over j -- row 64 is the softmax
           denominator (ones column). Normalize with reciprocal_approx_fast
           + gpsimd partition_broadcast.
  phase 3: out^T[f,t] = sum_p Wo_pair[d128,f].T @ O^T_pair[d128,t].
"""

import numpy as np

import concourse.bacc as bacc
import concourse.tile as tile
import concourse.mybir as mybir
from concourse import bass_utils
from concourse.bass import ts

F32 = mybir.dt.float32
F32R = mybir.dt.float32r
EXP = mybir.ActivationFunctionType.Exp

T = 2048
TT = 16          # t tiles of 128
NP = 4           # head pairs per core
NQC = 4          # q chunks of 512
SCALE = 0.125    # 1/sqrt(64)

_CACHE = {}
_last_in_maps = None


def _build(CT):
    """CT = number of 128-row c-tiles in the (possibly bias-augmented) x/W."""
    nc = bacc.Bacc("TRN2", target_bir_lowering=False, debug=False)
    C = CT * 128

    # keep PE instructions in emission order: the scheduler otherwise
    # interleaves S/AV/proj matmuls, paying a ~250ns array-reconfig penalty
    # on every K=64 <-> K=128 transition
    from concourse.bass import _add_dep_helper

    _pe_last = [None]

    def _chain(inst):
        _pe_last[0] = inst
        return inst

    def mm(*args, **kwargs):
        return _chain(nc.tensor.matmul(*args, **kwargs))

    def mmt(*args, **kwargs):
        return _chain(nc.tensor.transpose(*args, **kwargs))

    xa = nc.dram_tensor("xa", [T, C], F32, kind="ExternalInput").ap()
    wq = nc.dram_tensor("wq", [C, 512], F32, kind="ExternalInput").ap()
    wk = nc.dram_tensor("wk", [C, 512], F32, kind="ExternalInput").ap()
    wv = nc.dram_tensor("wv", [C, 512], F32, kind="ExternalInput").ap()
    wo = nc.dram_tensor("wo", [512, 1024], F32, kind="ExternalInput").ap()
    tri = nc.dram_tensor("tri", [128, 128], F32, kind="ExternalInput").ap()
    idn = nc.dram_tensor("idn", [128, 128], F32, kind="ExternalInput").ap()
    ot = nc.dram_tensor("ot", [1024, T], F32, kind="ExternalOutput").ap()

    with tile.TileContext(nc) as tc:
        with (
            tc.tile_pool(name="persist", bufs=1) as persist,
            tc.tile_pool(name="dram", bufs=1, space="DRAM") as dpool,
        ):
            vS = persist.tile([128, TT, 8, 65], F32R)     # [k128, ktile, head, d+1]
            OT = persist.tile([128, NP, T], F32R)         # [d128(2 heads), pair, t]
            tr = persist.tile([128, 128], F32R)
            wo_sb = persist.tile([128, NP, 1024], F32R)
            nc.vector.memset(vS[:, :, :, 64:65].bitcast(F32), 1.0)

            # Q^T / K^T bounce chunks, one DRAM tile per (pair, t-chunk) so a
            # chunk becomes readable as soon as its projection lands
            qtd = {}
            ktd = {}
            for _p in range(NP):
                for _tc in range(4):
                    qtd[(_p, _tc)] = dpool.tile([128, 512], F32, name=f"qtd{_p}{_tc}")
                    ktd[(_p, _tc)] = dpool.tile([128, 512], F32, name=f"ktd{_p}{_tc}")

            # ---------------- phase 1: transpose + projections ----------------
            with (
                tc.tile_pool(name="ph1", bufs=1) as ph1,
                tc.tile_pool(name="xnat", bufs=6) as xnat,
                tc.tile_pool(name="bounce", bufs=4) as bpool,
                tc.tile_pool(name="pst", bufs=2, space="PSUM") as pst,
                tc.tile_pool(name="psp", bufs=6, space="PSUM") as psp,
            ):
                ident = ph1.tile([128, 128], F32)
                nc.sync.dma_start(out=ident, in_=idn)
                wv_sb = ph1.tile([128, CT, 512], F32R)
                wq_sb = ph1.tile([128, CT, NP, 128], F32R)
                wk_sb = ph1.tile([128, CT, NP, 128], F32R)
                xT = [ph1.tile([128, T], F32R, name=f"xT{cc}") for cc in range(CT)]

                def load_weights():
                    # big strided loads on the gpsimd queue set, emitted after
                    # the first transpose batch so x tiles go out first
                    nc.gpsimd.dma_start(out=tr, in_=tri.bitcast(F32R))
                    nc.gpsimd.dma_start(
                        out=wv_sb,
                        in_=wv.rearrange("(ct P) f -> P ct f", P=128).bitcast(F32R),
                    )
                    nc.gpsimd.dma_start(
                        out=wq_sb,
                        in_=wq.rearrange("(ct P) (np f) -> P ct np f", P=128, np=NP).bitcast(F32R),
                    )
                    nc.gpsimd.dma_start(
                        out=wk_sb,
                        in_=wk.rearrange("(ct P) (np f) -> P ct np f", P=128, np=NP).bitcast(F32R),
                    )
                    nc.gpsimd.dma_start(
                        out=wo_sb,
                        in_=wo.rearrange("(np P) f -> P np f", P=128).bitcast(F32R),
                    )

                def transpose_tt(tt):
                    for cc in range(CT):
                        xn = xnat.tile([128, 128], F32)
                        nc.sync.dma_start(out=xn, in_=xa[ts(tt, 128), ts(cc, 128)])
                        pt_ = pst.tile([128, 512], F32)
                        mmt(pt_[:, :128], xn, ident)
                        nc.vector.tensor_copy(out=xT[cc][:, ts(tt, 128)], in_=pt_[:, :128])

                def vproj_tt(tt):
                    ps = psp.tile([128, 512], F32)
                    for cc in range(CT):
                        mm(
                            ps,
                            lhsT=xT[cc][:, ts(tt, 128)],
                            rhs=wv_sb[:, cc, :],
                            start=(cc == 0),
                            stop=(cc == CT - 1),
                        )
                    nc.vector.tensor_copy(
                        out=vS[:, tt, :, 0:64],
                        in_=ps.rearrange("p (h d) -> p h d", h=8),
                    )

                def qkproj_tc(tc_):
                    for p in range(NP):
                        for w_sb, dst, scl in ((wq_sb, qtd, SCALE), (wk_sb, ktd, 1.0)):
                            ps = psp.tile([128, 512], F32)
                            for cc in range(CT):
                                mm(
                                    ps,
                                    lhsT=w_sb[:, cc, p, :],
                                    rhs=xT[cc][:, ts(tc_, 512)],
                                    start=(cc == 0),
                                    stop=(cc == CT - 1),
                                )
                            bo = bpool.tile([128, 512], F32)
                            nc.scalar.mul(out=bo, in_=ps, mul=scl)
                            nc.sync.dma_start(out=dst[(p, tc_)], in_=bo)

                for tt in range(TT + 1):
                    if tt < TT:
                        transpose_tt(tt)
                    if tt == 0:
                        load_weights()
                    if tt >= 1:
                        vproj_tt(tt - 1)
                        if (tt - 1) % 4 == 3:
                            qkproj_tc((tt - 1) // 4)

            # ---------------- phase 2: attention ----------------
            # Emission keeps the PE in same-type runs: a group of 3 j-steps of
            # S matmuls (+exp on ACT), then the previous group's AV matmuls.
            # Interleaving S/AV per-j costs ~25%/MM in PE streaming rate.
            with (
                tc.tile_pool(name="qkc", bufs=12) as qkcpool,
                tc.tile_pool(name="ptp", bufs=12) as ptpool,
                tc.tile_pool(name="rsm", bufs=4) as rpool,
                tc.tile_pool(name="rbcp", bufs=4) as rbcpool,
                tc.tile_pool(name="psS", bufs=3, space="PSUM") as psS,
                tc.tile_pool(name="psAv", bufs=2, space="PSUM") as psAv,
            ):
                qch = {}
                kch = {}
                avs = {}
                pts = {}

                def fetch_pair(p):
                    for tc_ in range(4):
                        qt = qkcpool.tile([128, 512], F32R, name="qTc", tag="qTc")
                        nc.sync.dma_start(out=qt, in_=qtd[(p, tc_)].bitcast(F32R))
                        qch[(p, tc_)] = qt
                        kt = qkcpool.tile([128, 512], F32R, name="kTc", tag="kTc")
                        nc.sync.dma_start(out=kt, in_=ktd[(p, tc_)].bitcast(F32R))
                        kch[(p, tc_)] = kt

                def s_exp(p, qc, j):
                    off = max(0, 128 * j - 512 * qc)
                    sg = psS.tile([128, 2, 512], F32)
                    kc = kch[(p, j // 4)]
                    qc_t = qch[(p, qc)]
                    jo = 128 * (j % 4)
                    for m in range(2):
                        mm(
                            sg[:, m, off:],
                            lhsT=kc[64 * m : 64 * m + 64, jo : jo + 128],
                            rhs=qc_t[64 * m : 64 * m + 64, off:],
                            start=True,
                            stop=True,
                        )
                    ptile = ptpool.tile([128, 2, 512], F32R)
                    nc.scalar.activation(
                        out=ptile[:, :, off:], in_=sg[:, :, off:], func=EXP
                    )
                    if j >= 4 * qc:
                        nc.vector.tensor_mul(
                            ptile[:, :, off : off + 128],
                            ptile[:, :, off : off + 128],
                            tr[:, None, :].to_broadcast([128, 2, 128]),
                        )
                    pts[(p, qc, j)] = (ptile, off)

                def av_mm(p, qc, j, nj):
                    ptile, off = pts.pop((p, qc, j))
                    av = avs[(p, qc)]
                    for m in range(2):
                        mm(
                            av[m][:65, off:],
                            lhsT=vS[:, j, 2 * p + m, :],
                            rhs=ptile[:, m, off:],
                            start=(j == 0),
                            stop=(j == nj - 1),
                        )

                def normalize(p, qc):
                    av = avs.pop((p, qc))
                    rsbs = []
                    for m in range(2):
                        rsb = rpool.tile([1, 512], F32, name="rsb", tag="rsb")
                        nc.vector.tensor_copy(out=rsb, in_=av[m][64:65, :])
                        # unnormalized O~ out of PSUM so the av bank frees fast
                        nc.vector.tensor_copy(
                            out=OT[64 * m : 64 * m + 64, p, ts(qc, 512)],
                            in_=av[m][0:64, :],
                        )
                        rsbs.append(rsb)
                    for m in range(2):
                        rinv = rpool.tile([1, 512], F32, name="rinv", tag="rinv")
                        nc.vector.reciprocal_approx_fast(out=rinv, in_=rsbs[m])
                        rb = rbcpool.tile([128, 512], F32, name="rb", tag="rb")
                        nc.gpsimd.partition_broadcast(rb, rinv)
                        sl = OT[64 * m : 64 * m + 64, p, ts(qc, 512)]
                        nc.vector.tensor_mul(sl, sl, rb[64 * m : 64 * m + 64, :])

                groups = []
                for p in range(NP):
                    for qc in range(NQC):
                        nj = 4 * qc + 4
                        js = list(range(nj))
                        sub = [js[i : i + 3] for i in range(0, nj, 3)]
                        for gi, jg in enumerate(sub):
                            groups.append((p, qc, nj, jg, gi == 0, gi == len(sub) - 1))

                def av_group(gi):
                    p, qc, nj, jg, first, last = groups[gi]
                    if first:
                        avs[(p, qc)] = [
                            psAv.tile([128, 512], F32, name="av", tag="av")
                            for _ in range(2)
                        ]
                    for j in jg:
                        av_mm(p, qc, j, nj)
                    if last:
                        normalize(p, qc)

                # S-runs of 6 MMs; AV-runs of ~12 (two groups) to amortize the
                # PE row-config switch between K=64 S and K=128 AV matmuls
                LAG = 2
                for i in range(len(groups) + LAG):
                    if i < len(groups):
                        p, qc, nj, jg, first, last = groups[i]
                        if qc == 0 and first:
                            fetch_pair(p)
                        for j in jg:
                            s_exp(p, qc, j)
                    if i >= LAG and (i - LAG) % 2 == 1:
                        av_group(i - LAG - 1)
                        av_group(i - LAG)
                if len(groups) % 2 == 1:
                    av_group(len(groups) - 1)

            # ---------------- phase 3: output projection ----------------
            with (
                tc.tile_pool(name="obnc", bufs=4) as opool,
                tc.tile_pool(name="psO", bufs=8, space="PSUM") as psO,
            ):
                for ft in range(8):
                    pso = [psO.tile([128, 512], F32, name="pso", tag="pso") for _ in range(4)]
                    for p in range(NP):
                        for tc_ in range(4):
                            mm(
                                pso[tc_],
                                lhsT=wo_sb[:, p, ts(ft, 128)],
                                rhs=OT[:, p, ts(tc_, 512)],
                                start=(p == 0),
                                stop=(p == NP - 1),
                            )
                    for tc_ in range(4):
                        ob = opool.tile([128, 512], F32)
                        nc.vector.tensor_copy(out=ob, in_=pso[tc_])
                        nc.sync.dma_start(out=ot[ts(ft, 128), ts(tc_, 512)], in_=ob)

    nc.compile()
    return nc


def kernel(x, W_qkv, b_qkv, W_out, b_out):
    global _last_in_maps
    x = np.asarray(x, dtype=np.float32)
    W_qkv = np.asarray(W_qkv, dtype=np.float32)
    b_qkv = np.asarray(b_qkv, dtype=np.float32)
    W_out = np.asarray(W_out, dtype=np.float32)
    b_out = np.asarray(b_out, dtype=np.float32)
    B = x.shape[0]

    aug = bool(np.any(b_qkv))
    CT = 9 if aug else 8
    if CT not in _CACHE:
        _CACHE[CT] = _build(CT)
    nc = _CACHE[CT]

    # triangle keep-mask for the diagonal 128 block: [p, c] = 1 if c >= p
    tri = (np.arange(128)[None, :] >= np.arange(128)[:, None]).astype(np.float32)

    in_maps = []
    for core in range(8):
        b, g = core // 2, core % 2
        xa = x[b]
        if aug:
            pad = np.zeros((T, 128), np.float32)
            pad[:, 0] = 1.0
            xa = np.concatenate([xa, pad], axis=1)

        def wslice(col0):
            w = W_qkv[:, col0 + 512 * g : col0 + 512 * g + 512]
            if aug:
                extra = np.zeros((128, 512), np.float32)
                extra[0] = b_qkv[col0 + 512 * g : col0 + 512 * g + 512]
                w = np.concatenate([w, extra], axis=0)
            return np.ascontiguousarray(w)

        in_maps.append(
            {
                "xa": np.ascontiguousarray(xa),
                "wq": wslice(0),
                "wk": wslice(1024),
                "wv": wslice(2048),
                "wo": np.ascontiguousarray(W_out[512 * g : 512 * g + 512, :]),
                "tri": tri,
                "idn": np.eye(128, dtype=np.float32),
            }
        )

    _last_in_maps = in_maps
    res = bass_utils.run_bass_kernel_spmd(nc, in_maps, list(range(8))).results
    out = np.empty((B, T, 1024), np.float32)
    for b in range(B):
        acc = res[2 * b]["ot"] + res[2 * b + 1]["ot"]
        out[b] = acc.T + b_out[None, :]
    return out



# revision 4
# speedup vs baseline: 1.2154x; 1.2154x over previous
"""Causal self-attention (B=4, T=2048, D=1024, H=16) on 8 TRN2 NeuronCores.

Sharding: core i = (batch b = i//2, head-group g = i%2). Data parallel on B,
tensor parallel on heads (8 heads per group): qkv_proj columns and out_proj
rows split per head group. Each core computes a partial [D, T] output^T for
its batch; host sums the two group partials per batch, transposes, adds bias.

v2 design (vs v1): everything bf16 on the PE (fp32 PSUM accumulation), Q^T/K^T
kept resident in SBUF (no DRAM bounce), and a single fused emission stream:
transposes, V/Q/K projections and the output projection are lazily interleaved
between attention S/AV groups so the PE never idles while the scalar engine
(exp) chews through softmax. The 1/sqrt(dh) scale is folded into W_q on host.
"""

import numpy as np
import ml_dtypes

import concourse.bacc as bacc
import concourse.tile as tile
import concourse.mybir as mybir
from concourse import bass_utils
from concourse.bass import ts

F32 = mybir.dt.float32
BF = mybir.dt.bfloat16
EXP = mybir.ActivationFunctionType.Exp

T = 2048
TT = 16          # t tiles of 128
NP = 4           # head pairs per core
NQC = 4          # q chunks of 512

_CACHE = {}
_last_in_maps = None


def _build(CT):
    """CT = number of 128-row c-tiles in the (possibly bias-augmented) x/W."""
    nc = bacc.Bacc("TRN2", target_bir_lowering=False, debug=False)
    C = CT * 128

    xa = nc.dram_tensor("xa", [T, C], BF, kind="ExternalInput").ap()
    wq = nc.dram_tensor("wq", [C, 512], BF, kind="ExternalInput").ap()
    wk = nc.dram_tensor("wk", [C, 512], BF, kind="ExternalInput").ap()
    wv = nc.dram_tensor("wv", [C, 512], BF, kind="ExternalInput").ap()
    wo = nc.dram_tensor("wo", [512, 1024], BF, kind="ExternalInput").ap()
    tri = nc.dram_tensor("tri", [128, 128], BF, kind="ExternalInput").ap()
    idn = nc.dram_tensor("idn", [128, 128], BF, kind="ExternalInput").ap()
    ot = nc.dram_tensor("ot", [1024, T], F32, kind="ExternalOutput").ap()

    mm = None  # set below

    with tile.TileContext(nc) as tc:
        with (
            tc.tile_pool(name="persist", bufs=1) as persist,
        ):
            mm = nc.tensor.matmul
            mmt = nc.tensor.transpose

            # resident tensors (per-partition bytes in comments)
            xT = persist.tile([128, CT, T], BF)            # 32K (CT=8)
            QK = persist.tile([128, NP, 2, NQC, 512], BF)  # 32K  [q=0/k=1]
            vS = persist.tile([128, TT, 8, 65], BF)        # 16.6K [k,tt,head,d+1]
            OT = persist.tile([128, NP, T], BF)            # 16.4K
            wq_sb = persist.tile([128, CT, NP, 128], BF)   # 8.2K
            wk_sb = persist.tile([128, CT, NP, 128], BF)   # 8.2K
            wv_sb = persist.tile([128, CT, 512], BF)       # 8.2K
            wo_sb = persist.tile([128, NP, 1024], BF)      # 8.2K
            tr = persist.tile([128, 128], BF)
            ident = persist.tile([128, 128], BF)
            nc.vector.memset(vS[:, :, :, 64:65], 1.0)
            nc.sync.dma_start(out=ident, in_=idn)

            with (
                tc.tile_pool(name="xnat", bufs=8) as xnat,
                tc.tile_pool(name="ptp", bufs=10) as ptpool,
                tc.tile_pool(name="rsm", bufs=6) as rpool,
                tc.tile_pool(name="rbcp", bufs=4) as rbcpool,
                tc.tile_pool(name="obnc", bufs=4) as opool,
                tc.tile_pool(name="psS", bufs=2, space="PSUM") as psS,      # 4 banks
                tc.tile_pool(name="psAv", bufs=2, space="PSUM") as psAv,    # 2 banks
                tc.tile_pool(name="pso", bufs=2, space="PSUM") as psO,      # 2 banks
            ):
                # ---------- setup work units (emitted lazily) ----------
                def load_weights():
                    nc.gpsimd.dma_start(out=tr, in_=tri)
                    nc.gpsimd.dma_start(
                        out=wv_sb,
                        in_=wv.rearrange("(ct P) f -> P ct f", P=128),
                    )
                    nc.gpsimd.dma_start(
                        out=wq_sb,
                        in_=wq.rearrange("(ct P) (np f) -> P ct np f", P=128, np=NP),
                    )
                    nc.gpsimd.dma_start(
                        out=wk_sb,
                        in_=wk.rearrange("(ct P) (np f) -> P ct np f", P=128, np=NP),
                    )

                def load_wo():
                    nc.gpsimd.dma_start(
                        out=wo_sb,
                        in_=wo.rearrange("(np P) f -> P np f", P=128),
                    )

                def transpose_tt(tt):
                    # quads of 4 transposes share one psum tile; one DVE copy
                    for q4 in range(0, CT, 4):
                        ncc = min(4, CT - q4)
                        pt_ = psO.tile([128, 512], F32, name="pso", tag="pso")
                        for k in range(ncc):
                            cc = q4 + k
                            xn = xnat.tile([128, 128], BF)
                            nc.sync.dma_start(
                                out=xn, in_=xa[ts(tt, 128), ts(cc, 128)]
                            )
                            mmt(
                                pt_[:, 64 * k : 64 * k + 64].bitcast(BF),
                                xn,
                                ident,
                            )
                        nc.vector.tensor_copy(
                            out=xT[:, q4 : q4 + ncc, ts(tt, 128)],
                            in_=pt_[:, 0 : 64 * ncc]
                            .bitcast(BF)
                            .rearrange("p (c t) -> p c t", c=ncc),
                        )

                def vproj_tt(tt):
                    ps = psO.tile([128, 512], F32, name="pso", tag="pso")
                    for cc in range(CT):
                        mm(
                            ps,
                            lhsT=xT[:, cc, ts(tt, 128)],
                            rhs=wv_sb[:, cc, :],
                            start=(cc == 0),
                            stop=(cc == CT - 1),
                        )
                    nc.vector.tensor_copy(
                        out=vS[:, tt, :, 0:64],
                        in_=ps.rearrange("p (h d) -> p h d", h=8),
                    )

                def proj_chunk(p, kind, tc_):
                    # kind: 0 = q, 1 = k
                    w_sb = wq_sb if kind == 0 else wk_sb
                    ps = psO.tile([128, 512], F32, name="pso", tag="pso")
                    for cc in range(CT):
                        mm(
                            ps,
                            lhsT=w_sb[:, cc, p, :],
                            rhs=xT[:, cc, ts(tc_, 512)],
                            start=(cc == 0),
                            stop=(cc == CT - 1),
                        )
                    nc.vector.tensor_copy(out=QK[:, p, kind, tc_, :], in_=ps)

                def phase3_chunk(tc_):
                    for ft in range(8):
                        ps = psO.tile([128, 512], F32, name="pso", tag="pso")
                        for p in range(NP):
                            mm(
                                ps,
                                lhsT=wo_sb[:, p, ts(ft, 128)],
                                rhs=OT[:, p, ts(tc_, 512)],
                                start=(p == 0),
                                stop=(p == NP - 1),
                            )
                        ob = opool.tile([128, 512], F32)
                        nc.vector.tensor_copy(out=ob, in_=ps)
                        nc.sync.dma_start(out=ot[ts(ft, 128), ts(tc_, 512)], in_=ob)

                # lazy emission bookkeeping
                done_tt = [0]
                done_proj = set()
                fillers = []

                def need_tt(up_to):
                    while done_tt[0] < min(up_to, TT):
                        tt = done_tt[0]
                        transpose_tt(tt)
                        if tt == 0:
                            load_weights()
                        vproj_tt(tt)
                        done_tt[0] += 1

                def need_proj(p, kind, tc_):
                    if p >= NP or tc_ >= NQC:
                        return
                    key = (p, kind, tc_)
                    if key in done_proj:
                        return
                    done_proj.add(key)
                    proj_chunk(p, kind, tc_)

                def pop_filler():
                    if fillers:
                        fillers.pop(0)()

                # ---------- attention work ----------
                qch = lambda p, tc_: QK[:, p, 0, tc_, :]
                kch = lambda p, tc_: QK[:, p, 1, tc_, :]
                avs = {}
                pts = {}
                SCALE_DONE = True  # 1/sqrt(dh) folded into wq on host

                def s_exp(p, qc, j):
                    off = max(0, 128 * j - 512 * qc)
                    sg = psS.tile([128, 2, 512], F32)
                    kc = kch(p, j // 4)
                    qc_t = qch(p, qc)
                    jo = 128 * (j % 4)
                    for m in range(2):
                        mm(
                            sg[:, m, off:],
                            lhsT=kc[64 * m : 64 * m + 64, jo : jo + 128],
                            rhs=qc_t[64 * m : 64 * m + 64, off:],
                            start=True,
                            stop=True,
                        )
                    ptile = ptpool.tile([128, 2, 512], BF)
                    nc.scalar.activation(
                        out=ptile[:, :, off:], in_=sg[:, :, off:], func=EXP
                    )
                    if j >= 4 * qc:
                        nc.vector.tensor_mul(
                            ptile[:, :, off : off + 128],
                            ptile[:, :, off : off + 128],
                            tr[:, None, :].to_broadcast([128, 2, 128]),
                        )
                    pts[(p, qc, j)] = (ptile, off)

                def av_mm(p, qc, j, nj):
                    ptile, off = pts.pop((p, qc, j))
                    av = avs[(p, qc)]
                    for m in range(2):
                        mm(
                            av[m][:65, off:],
                            lhsT=vS[:, j, 2 * p + m, :],
                            rhs=ptile[:, m, off:],
                            start=(j == 0),
                            stop=(j == nj - 1),
                        )

                def normalize(p, qc):
                    av = avs.pop((p, qc))
                    rsbs = []
                    for m in range(2):
                        rsb = rpool.tile([1, 512], F32, name="rsb", tag="rsb")
                        nc.vector.tensor_copy(out=rsb, in_=av[m][64:65, :])
                        # unnormalized O~ out of PSUM so the av bank frees fast
                        nc.vector.tensor_copy(
                            out=OT[64 * m : 64 * m + 64, p, ts(qc, 512)],
                            in_=av[m][0:64, :],
                        )
                        rsbs.append(rsb)
                    for m in range(2):
                        rinv = rpool.tile([1, 512], F32, name="rinv", tag="rinv")
                        nc.vector.reciprocal_approx_fast(out=rinv, in_=rsbs[m])
                        rinv_b = rpool.tile([1, 512], BF, name="rinvb", tag="rinvb")
                        nc.vector.tensor_copy(out=rinv_b, in_=rinv)
                        rb = rbcpool.tile([128, 512], BF, name="rb", tag="rb")
                        nc.gpsimd.partition_broadcast(rb, rinv_b)
                        sl = OT[64 * m : 64 * m + 64, p, ts(qc, 512)]
                        nc.vector.tensor_mul(sl, sl, rb[64 * m : 64 * m + 64, :])
                    if p == NP - 1:
                        # output projection for this t-chunk can go now
                        fillers.insert(0, lambda qc=qc: phase3_chunk(qc))

                # group schedule (same S-run/AV-run LAG structure as v1)
                groups = []
                for p in range(NP):
                    for qc in range(NQC):
                        nj = 4 * qc + 4
                        js = list(range(nj))
                        sub = [js[i : i + 3] for i in range(0, nj, 3)]
                        for gi, jg in enumerate(sub):
                            groups.append(
                                (p, qc, nj, jg, gi == 0, gi == len(sub) - 1)
                            )

                def av_group(gi):
                    p, qc, nj, jg, first, last = groups[gi]
                    if first:
                        avs[(p, qc)] = [
                            psAv.tile([128, 512], F32, name="av", tag="av")
                            for _ in range(2)
                        ]
                    for j in jg:
                        av_mm(p, qc, j, nj)
                    if last:
                        normalize(p, qc)

                # prologue: enough setup for block (0,0)
                need_tt(4)
                need_proj(0, 1, 0)
                need_proj(0, 0, 0)

                LAG = 2
                for i in range(len(groups) + LAG):
                    if i < len(groups):
                        p, qc, nj, jg, first, last = groups[i]
                        if first:
                            # hard deps for this block (usually no-ops)
                            need_tt(4 * qc + 4)
                            need_proj(p, 1, qc)
                            need_proj(p, 0, qc)
                            # soft prefetches, spread between groups
                            if p == 0:
                                fillers.append(lambda u=4 * qc + 8: need_tt(u))
                                fillers.append(
                                    lambda a=qc + 1: (
                                        need_proj(0, 1, a),
                                        need_proj(0, 0, a),
                                    )
                                )
                            fillers.append(
                                lambda b=p + 1, a=qc: (
                                    need_proj(b, 1, a),
                                    need_proj(b, 0, a),
                                )
                            )
                            if p == 1 and qc == 3:
                                fillers.append(load_wo)
                        for j in jg:
                            s_exp(p, qc, j)
                        pop_filler()
                    if i >= LAG and (i - LAG) % 2 == 1:
                        av_group(i - LAG - 1)
                        av_group(i - LAG)
                        pop_filler()
                if len(groups) % 2 == 1:
                    av_group(len(groups) - 1)
                while fillers:
                    pop_filler()

    nc.compile()
    return nc


def kernel(x, W_qkv, b_qkv, W_out, b_out):
    global _last_in_maps
    x = np.asarray(x, dtype=np.float32)
    W_qkv = np.asarray(W_qkv, dtype=np.float32)
    b_qkv = np.asarray(b_qkv, dtype=np.float32)
    W_out = np.asarray(W_out, dtype=np.float32)
    b_out = np.asarray(b_out, dtype=np.float32)
    B = x.shape[0]
    BF_NP = ml_dtypes.bfloat16

    aug = bool(np.any(b_qkv))
    CT = 9 if aug else 8
    if CT not in _CACHE:
        _CACHE[CT] = _build(CT)
    nc = _CACHE[CT]

    # triangle keep-mask for the diagonal 128 block: [p, c] = 1 if c >= p
    tri = (np.arange(128)[None, :] >= np.arange(128)[:, None]).astype(BF_NP)
    SCALE = 1.0 / np.sqrt(64.0)

    in_maps = []
    for core in range(8):
        b, g = core // 2, core % 2
        xa = x[b]
        if aug:
            pad = np.zeros((T, 128), np.float32)
            pad[:, 0] = 1.0
            xa = np.concatenate([xa, pad], axis=1)

        def wslice(col0, scale=1.0):
            w = W_qkv[:, col0 + 512 * g : col0 + 512 * g + 512]
            if aug:
                extra = np.zeros((128, 512), np.float32)
                extra[0] = b_qkv[col0 + 512 * g : col0 + 512 * g + 512]
                w = np.concatenate([w, extra], axis=0)
            return np.ascontiguousarray(w * scale).astype(BF_NP)

        in_maps.append(
            {
                "xa": np.ascontiguousarray(xa).astype(BF_NP),
                "wq": wslice(0, SCALE),
                "wk": wslice(1024),
                "wv": wslice(2048),
                "wo": np.ascontiguousarray(
                    W_out[512 * g : 512 * g + 512, :]
                ).astype(BF_NP),
                "tri": tri,
                "idn": np.eye(128, dtype=BF_NP),
            }
        )

    _last_in_maps = in_maps
    res = bass_utils.run_bass_kernel_spmd(nc, in_maps, list(range(8))).results
    out = np.empty((B, T, 1024), np.float32)
    for b in range(B):
        acc = res[2 * b]["ot"].astype(np.float32) + res[2 * b + 1]["ot"].astype(
            np.float32
        )
        out[b] = acc.T + b_out[None, :]
    return out


# revision 9
# speedup vs baseline: 1.3460x; 1.1074x over previous
"""Causal self-attention (B=4, T=2048, D=1024, H=16) on 8 TRN2 NeuronCores.

Sharding: core i = (batch b = i//2, head-group g = i%2). Data parallel on B,
tensor parallel on heads (8 heads per group): qkv_proj columns and out_proj
rows split per head group. Each core computes a partial [D, T] output^T for
its batch; host sums the two group partials per batch, transposes, adds bias.

v3 design: all-bf16 PE datapath (fp32 PSUM accumulation), Q^T/K^T resident in
SBUF, and a qc-major fused schedule: attention blocks run (qc, pair) ordered so
each q-chunk level only needs transposes/projections for t <= 512*(qc+1);
setup work (x transpose, V/Q/K projections), the output projection and DMAs
are lazily interleaved between S/AV groups to keep the PE busy while the
scalar engine streams exp(). The 1/sqrt(dh) scale is folded into W_q on host.
Elementwise load is split: exp on Scalar, normalize/copies on Vector, causal
mask-mul + Q/K psum casts on GpSimd.
"""

import numpy as np
import ml_dtypes

import concourse.bacc as bacc
import concourse.tile as tile
import concourse.mybir as mybir
from concourse import bass_utils
from concourse.bass import ts

F32 = mybir.dt.float32
BF = mybir.dt.bfloat16
EXP = mybir.ActivationFunctionType.Exp

T = 2048
TT = 16          # t tiles of 128
NP = 4           # head pairs per core
NQC = 4          # q chunks of 512

_CACHE = {}
_last_in_maps = None


def _build(CT):
    """CT = number of 128-row c-tiles in the (possibly bias-augmented) x/W."""
    nc = bacc.Bacc("TRN2", target_bir_lowering=False, debug=False)
    C = CT * 128

    xa = nc.dram_tensor("xa", [T, C], BF, kind="ExternalInput").ap()
    wq = nc.dram_tensor("wq", [C, 512], BF, kind="ExternalInput").ap()
    wk = nc.dram_tensor("wk", [C, 512], BF, kind="ExternalInput").ap()
    wv = nc.dram_tensor("wv", [C, 512], BF, kind="ExternalInput").ap()
    wo = nc.dram_tensor("wo", [512, 1024], BF, kind="ExternalInput").ap()
    tri = nc.dram_tensor("tri", [128, 128], BF, kind="ExternalInput").ap()
    idn = nc.dram_tensor("idn", [128, 128], BF, kind="ExternalInput").ap()
    ot = nc.dram_tensor("ot", [1024, T], F32, kind="ExternalOutput").ap()

    with tile.TileContext(nc) as tc:
        with (
            tc.tile_pool(name="persist", bufs=1) as persist,
        ):
            mm = nc.tensor.matmul
            mmt = nc.tensor.transpose

            # resident tensors (per-partition bytes in comments)
            xT = persist.tile([128, CT, T], BF)            # 32K (CT=8)
            QK = persist.tile([128, NP, 2, NQC, 512], BF)  # 32K  [q=0/k=1]
            vS = persist.tile([128, TT, 8, 65], BF)        # 16.6K [k,tt,head,d+1]
            OT = persist.tile([128, NP, T], BF)            # 16.4K
            wq_sb = persist.tile([128, CT, NP, 128], BF)   # 8.2K
            wk_sb = persist.tile([128, CT, NP, 128], BF)   # 8.2K
            wv_sb = persist.tile([128, CT, 512], BF)       # 8.2K
            wo_sb = persist.tile([128, NP, 1024], BF)      # 8.2K
            tr = persist.tile([128, 128], BF)
            ident = persist.tile([128, 128], BF)
            nc.vector.memset(vS[:, :, :, 64:65], 1.0)
            nc.sync.dma_start(out=ident, in_=idn)

            with (
                tc.tile_pool(name="xnat", bufs=4) as xnat,
                tc.tile_pool(name="ptp", bufs=10) as ptpool,
                tc.tile_pool(name="rsm", bufs=6) as rpool,
                tc.tile_pool(name="rbcp", bufs=4) as rbcpool,
                tc.tile_pool(name="obnc", bufs=4) as opool,
                tc.tile_pool(name="psS", bufs=2, space="PSUM") as psS,      # 4 banks
                tc.tile_pool(name="psAv", bufs=2, space="PSUM") as psAv,    # 2 banks
                tc.tile_pool(name="pso", bufs=2, space="PSUM") as psO,      # 2 banks
            ):
                # ---------- setup work units (emitted lazily) ----------
                def load_weights():
                    nc.gpsimd.dma_start(out=tr, in_=tri)
                    nc.gpsimd.dma_start(
                        out=wv_sb,
                        in_=wv.rearrange("(ct P) f -> P ct f", P=128),
                    )
                    nc.gpsimd.dma_start(
                        out=wq_sb,
                        in_=wq.rearrange("(ct P) (np f) -> P ct np f", P=128, np=NP),
                    )
                    nc.gpsimd.dma_start(
                        out=wk_sb,
                        in_=wk.rearrange("(ct P) (np f) -> P ct np f", P=128, np=NP),
                    )

                def load_wo():
                    nc.gpsimd.dma_start(
                        out=wo_sb,
                        in_=wo.rearrange("(np P) f -> P np f", P=128),
                    )

                def transpose_tt(tt):
                    # one DMA per 128-token slab; transpose quads share a psum
                    # tile so one DVE copy moves 4 c-tiles into xT
                    xn = xnat.tile([128, C], BF)
                    nc.sync.dma_start(out=xn, in_=xa[ts(tt, 128), :])
                    for q4 in range(0, CT, 4):
                        ncc = min(4, CT - q4)
                        pt_ = psO.tile([128, 512], F32, name="pso", tag="pso")
                        for k in range(ncc):
                            cc = q4 + k
                            mmt(
                                pt_[:, 64 * k : 64 * k + 64].bitcast(BF),
                                xn[:, ts(cc, 128)],
                                ident,
                            )
                        nc.vector.tensor_copy(
                            out=xT[:, q4 : q4 + ncc, ts(tt, 128)],
                            in_=pt_[:, 0 : 64 * ncc]
                            .bitcast(BF)
                            .rearrange("p (c t) -> p c t", c=ncc),
                        )

                def vproj_tt(tt):
                    ps = psO.tile([128, 512], F32, name="pso", tag="pso")
                    for cc in range(CT):
                        mm(
                            ps,
                            lhsT=xT[:, cc, ts(tt, 128)],
                            rhs=wv_sb[:, cc, :],
                            start=(cc == 0),
                            stop=(cc == CT - 1),
                        )
                    nc.vector.tensor_copy(
                        out=vS[:, tt, :, 0:64],
                        in_=ps.rearrange("p (h d) -> p h d", h=8),
                    )

                def proj_chunk(p, kind, tc_):
                    # kind: 0 = q, 1 = k
                    w_sb = wq_sb if kind == 0 else wk_sb
                    ps = psO.tile([128, 512], F32, name="pso", tag="pso")
                    for cc in range(CT):
                        mm(
                            ps,
                            lhsT=w_sb[:, cc, p, :],
                            rhs=xT[:, cc, ts(tc_, 512)],
                            start=(cc == 0),
                            stop=(cc == CT - 1),
                        )
                    nc.vector.tensor_copy(out=QK[:, p, kind, tc_, :], in_=ps)

                def phase3_half(tc_, h):
                    for ft in range(4 * h, 4 * h + 4):
                        ps = psO.tile([128, 512], F32, name="pso", tag="pso")
                        for p in range(NP):
                            mm(
                                ps,
                                lhsT=wo_sb[:, p, ts(ft, 128)],
                                rhs=OT[:, p, ts(tc_, 512)],
                                start=(p == 0),
                                stop=(p == NP - 1),
                            )
                        ob = opool.tile([128, 512], F32)
                        nc.vector.tensor_copy(out=ob, in_=ps)
                        nc.sync.dma_start(out=ot[ts(ft, 128), ts(tc_, 512)], in_=ob)

                # lazy emission bookkeeping
                done_tt = [0]
                done_proj = set()
                fillers = []

                def need_tt(up_to):
                    while done_tt[0] < min(up_to, TT):
                        tt = done_tt[0]
                        transpose_tt(tt)
                        if tt == 0:
                            load_weights()
                            load_wo()
                        vproj_tt(tt)
                        done_tt[0] += 1

                def need_proj(p, kind, tc_):
                    if p >= NP or tc_ >= NQC:
                        return
                    key = (p, kind, tc_)
                    if key in done_proj:
                        return
                    done_proj.add(key)
                    proj_chunk(p, kind, tc_)

                def pop_filler():
                    if fillers:
                        fillers.pop(0)()

                # ---------- attention work ----------
                qch = lambda p, tc_: QK[:, p, 0, tc_, :]
                kch = lambda p, tc_: QK[:, p, 1, tc_, :]
                avs = {}
                pts = {}
                level_done = [0] * NQC

                def s_exp(p, qc, j):
                    off = max(0, 128 * j - 512 * qc)
                    sg = psS.tile([128, 2, 512], F32)
                    kc = kch(p, j // 4)
                    qc_t = qch(p, qc)
                    jo = 128 * (j % 4)
                    for m in range(2):
                        mm(
                            sg[:, m, off:],
                            lhsT=kc[64 * m : 64 * m + 64, jo : jo + 128],
                            rhs=qc_t[64 * m : 64 * m + 64, off:],
                            start=True,
                            stop=True,
                        )
                    ptile = ptpool.tile([128, 2, 512], BF)
                    nc.scalar.activation(
                        out=ptile[:, :, off:], in_=sg[:, :, off:], func=EXP
                    )
                    if j >= 4 * qc:
                        nc.vector.tensor_mul(
                            ptile[:, :, off : off + 128],
                            ptile[:, :, off : off + 128],
                            tr[:, None, :].to_broadcast([128, 2, 128]),
                        )
                    pts[(p, qc, j)] = (ptile, off)

                def av_mm(p, qc, j, nj):
                    ptile, off = pts.pop((p, qc, j))
                    av = avs[(p, qc)]
                    for m in range(2):
                        mm(
                            av[m][:65, off:],
                            lhsT=vS[:, j, 2 * p + m, :],
                            rhs=ptile[:, m, off:],
                            start=(j == 0),
                            stop=(j == nj - 1),
                        )

                def normalize(p, qc):
                    av = avs.pop((p, qc))
                    rsbs = []
                    for m in range(2):
                        rsb = rpool.tile([1, 512], F32, name="rsb", tag="rsb")
                        nc.vector.tensor_copy(out=rsb, in_=av[m][64:65, :])
                        # unnormalized O~ out of PSUM so the av bank frees fast
                        nc.vector.tensor_copy(
                            out=OT[64 * m : 64 * m + 64, p, ts(qc, 512)],
                            in_=av[m][0:64, :],
                        )
                        rsbs.append(rsb)
                    for m in range(2):
                        rinv = rpool.tile([1, 512], F32, name="rinv", tag="rinv")
                        nc.vector.reciprocal_approx_fast(out=rinv, in_=rsbs[m])
                        rinv_b = rpool.tile([1, 512], BF, name="rinvb", tag="rinvb")
                        nc.vector.tensor_copy(out=rinv_b, in_=rinv)
                        rb = rbcpool.tile([128, 512], BF, name="rb", tag="rb")
                        nc.gpsimd.partition_broadcast(rb, rinv_b)
                        sl = OT[64 * m : 64 * m + 64, p, ts(qc, 512)]
                        nc.vector.tensor_mul(sl, sl, rb[64 * m : 64 * m + 64, :])
                    level_done[qc] += 1
                    if level_done[qc] == NP:
                        # all pairs finished this t-chunk: output projection
                        fillers.insert(0, lambda a=qc: phase3_half(a, 0))
                        fillers.insert(1, lambda a=qc: phase3_half(a, 1))

                # groups: qc-major so each level only needs setup through
                # t = 512*(qc+1); pairs within a level share the causal shape
                groups = []
                for qc in range(NQC):
                    for p in range(NP):
                        nj = 4 * qc + 4
                        js = list(range(nj))
                        sub = [js[i : i + 3] for i in range(0, nj, 3)]
                        for gi, jg in enumerate(sub):
                            groups.append(
                                (p, qc, nj, jg, gi == 0, gi == len(sub) - 1)
                            )

                def av_group(gi):
                    p, qc, nj, jg, first, last = groups[gi]
                    if first:
                        avs[(p, qc)] = [
                            psAv.tile([128, 512], F32, name="av", tag="av")
                            for _ in range(2)
                        ]
                    for j in jg:
                        av_mm(p, qc, j, nj)
                    if last:
                        normalize(p, qc)

                # prologue: enough setup for block (p=0, qc=0)
                need_tt(4)
                need_proj(0, 1, 0)
                need_proj(0, 0, 0)

                LAG = 2
                for i in range(len(groups) + LAG):
                    if i < len(groups):
                        p, qc, nj, jg, first, last = groups[i]
                        if first:
                            # hard deps for this block (usually no-ops)
                            need_tt(4 * qc + 4)
                            need_proj(p, 1, qc)
                            need_proj(p, 0, qc)
                            # soft prefetches, spread between groups
                            if p == 0:
                                fillers.append(lambda u=4 * qc + 8: need_tt(u))
                            fillers.append(
                                lambda b=p, a=qc + 1: (
                                    need_proj(b, 1, a),
                                    need_proj(b, 0, a),
                                )
                            )
                        for j in jg:
                            s_exp(p, qc, j)
                        pop_filler()
                    if i >= LAG and (i - LAG) % 2 == 1:
                        av_group(i - LAG - 1)
                        av_group(i - LAG)
                        pop_filler()
                if len(groups) % 2 == 1:
                    av_group(len(groups) - 1)
                while fillers:
                    pop_filler()

    nc.compile()
    return nc


def kernel(x, W_qkv, b_qkv, W_out, b_out):
    global _last_in_maps
    x = np.asarray(x, dtype=np.float32)
    W_qkv = np.asarray(W_qkv, dtype=np.float32)
    b_qkv = np.asarray(b_qkv, dtype=np.float32)
    W_out = np.asarray(W_out, dtype=np.float32)
    b_out = np.asarray(b_out, dtype=np.float32)
    B = x.shape[0]
    BF_NP = ml_dtypes.bfloat16

    aug = bool(np.any(b_qkv))
    CT = 9 if aug else 8
    if CT not in _CACHE:
        _CACHE[CT] = _build(CT)
    nc = _CACHE[CT]

    # triangle keep-mask for the diagonal 128 block: [p, c] = 1 if c >= p
    tri = (np.arange(128)[None, :] >= np.arange(128)[:, None]).astype(BF_NP)
    SCALE = 1.0 / np.sqrt(64.0)

    in_maps = []
    for core in range(8):
        b, g = core // 2, core % 2
        xa = x[b]
        if aug:
            pad = np.zeros((T, 128), np.float32)
            pad[:, 0] = 1.0
            xa = np.concatenate([xa, pad], axis=1)

        def wslice(col0, scale=1.0):
            w = W_qkv[:, col0 + 512 * g : col0 + 512 * g + 512]
            if aug:
                extra = np.zeros((128, 512), np.float32)
                extra[0] = b_qkv[col0 + 512 * g : col0 + 512 * g + 512]
                w = np.concatenate([w, extra], axis=0)
            return np.ascontiguousarray(w * scale).astype(BF_NP)

        in_maps.append(
            {
                "xa": np.ascontiguousarray(xa).astype(BF_NP),
                "wq": wslice(0, SCALE),
                "wk": wslice(1024),
                "wv": wslice(2048),
                "wo": np.ascontiguousarray(
                    W_out[512 * g : 512 * g + 512, :]
                ).astype(BF_NP),
                "tri": tri,
                "idn": np.eye(128, dtype=BF_NP),
            }
        )

    _last_in_maps = in_maps
    res = bass_utils.run_bass_kernel_spmd(nc, in_maps, list(range(8))).results
    out = np.empty((B, T, 1024), np.float32)
    for b in range(B):
        acc = res[2 * b]["ot"].astype(np.float32) + res[2 * b + 1]["ot"].astype(
            np.float32
        )
        out[b] = acc.T + b_out[None, :]
    return out
